# revision 21
# baseline (speedup 1.0000x reference)
"""Causal self-attention Trainium2 kernel (v3: bf16 + fp8 S + phase overlap).

Full-model shapes: x [4, 2048, 1024], w_qkv [1024, 3072], b_qkv [3072],
w_out [1024, 1024], b_out [1024].  H=16 heads, D=64.

Sharding: 8 cores = 4 batches x 2 head-groups (tensor parallel).  Each core
computes qkv projection for its 8 heads of its batch, causal attention, and
the partial out-projection (512 of 1024 contraction rows).  The two partials
per batch are summed on the host (the "all-reduce" after out_proj), plus
b_out.

v3 strategy per core:
  - all inputs bf16 (x pre-transposed as xT [C, T], kept fully resident in
    SBUF so both head-groups' projections read it without re-DMA).
  - S matmuls run fp8e4m3 DoubleRow (0.5 cycles/row): q/k cast to fp8 by the
    phase-A PSUM->SBUF copies; the DoubleRow pair dim is a stride-0 broadcast
    of the same 64 contraction rows, so S is doubled and the 1/2 folds into
    the exp scale.  No K zero-padding.
  - exp in 1024-wide chunks (2 PSUM banks); u is bf16; PV/projections bf16.
  - softmax denominator via the ones-column of v; 1/Z on DVE straight from
    PSUM, GpSimd partition-broadcast, one DVE mul per (head, half) — all
    emitted eagerly per half.
  - the whole kernel is emitted as an interleaved schedule of attention
    "units" (head, t-half) and projection "fills" (qkv chunks of either
    group, out_proj tiles), so the PE works through projection chunks while
    ACT chews on exp during attention and neither engine idles long.
  - v-bias folded into the output as bvw = bv @ w_out; b_out added on host.
"""

import sys
from contextlib import ExitStack

import numpy as np

sys.path.insert(0, "/opt/trn_rl_repo")

import concourse.bacc as bacc
import concourse.bass as bass
import concourse.tile as tile
from concourse import mybir
from concourse.bass_utils import run_bass_kernel_spmd

F32 = mybir.dt.float32
BF16 = mybir.dt.bfloat16
FP8 = mybir.dt.float8e4

B, T, C, H = 4, 2048, 1024, 16
D = C // H  # 64
N_CORES = 8
HL = H // 2      # heads per core = 8
FL = HL * D      # local features = 512
G_HEADS = 4      # heads per inner group
G_F = G_HEADS * D  # 256

S_FP8 = True     # fp8 DoubleRow for the S matmul (else bf16)


def _chunks_for_s(t_start, tt, chunk=1024):
    out = []
    t0 = t_start
    while t0 < tt:
        w = min(chunk - (t0 % chunk), tt - t0)
        out.append((t0, w))
        t0 += w
    return out


def build_program(t_len=T):
    nc = bacc.Bacc(None, target_bir_lowering=False, debug=False)
    TT = t_len
    n_ttiles = TT // 128
    HW = min(1024, TT)  # t-half width
    n_halves = TT // HW
    qk_dt = FP8 if S_FP8 else BF16
    s_scale = (0.5 if S_FP8 else 1.0) / np.sqrt(D)
    KT = C // 128  # 8 contraction tiles

    xT = nc.declare_dram_parameter("xT", [C, TT], BF16, isOutput=False)
    wq = nc.declare_dram_parameter("wq", [C, FL], BF16, isOutput=False)
    wk = nc.declare_dram_parameter("wk", [C, FL], BF16, isOutput=False)
    wv = nc.declare_dram_parameter("wv", [C, FL], BF16, isOutput=False)
    wout = nc.declare_dram_parameter("wout", [FL, C], BF16, isOutput=False)
    bq = nc.declare_dram_parameter("bq", [FL], F32, isOutput=False)
    bk = nc.declare_dram_parameter("bk", [FL], F32, isOutput=False)
    bvw = nc.declare_dram_parameter("bvw", [1, C], F32, isOutput=False)
    tri = nc.declare_dram_parameter("tri", [128, 128], BF16, isOutput=False)
    tri8 = nc.declare_dram_parameter("tri8", [128, 128], FP8, isOutput=False)
    trineg = nc.declare_dram_parameter("trineg", [128, 128], F32, isOutput=False)
    vones = nc.declare_dram_parameter(
        "vones", [128, n_ttiles * G_HEADS], BF16, isOutput=False)
    out = nc.declare_dram_parameter("out", [TT, C], F32, isOutput=True)

    with tile.TileContext(nc) as tc, ExitStack() as top:
        const = top.enter_context(tc.tile_pool(name="const", bufs=1))
        persist = top.enter_context(tc.tile_pool(name="persist", bufs=1))
        u_pool = top.enter_context(tc.tile_pool(name="u", bufs=8))
        z_pool = top.enter_context(tc.tile_pool(name="z", bufs=2))

        # ---- constants ----
        tri_sb = const.tile([128, 128], BF16, name="tri_sb")
        tri8_sb = const.tile([128, 128], FP8, name="tri8_sb")
        trineg_sb = const.tile([128, 128], F32, name="trineg_sb")
        bq_sb = const.tile([128, FL // 128], F32, name="bq_sb")
        bk_sb = const.tile([128, FL // 128], F32, name="bk_sb")
        bvw_sb = const.tile([1, C], F32, name="bvw_sb")

        # ---- persistent SBUF state ----
        # full xT resident: [128, KT, TT] bf16 (32KB/partition)
        xsb = persist.tile([128, KT, TT], BF16, name="xsb")
        yT_sb = {}
        for j in range(FL // 128):
            for hf in range(n_halves):
                yT_sb[(j, hf)] = persist.tile(
                    [128, HW], BF16, tag=f"yT{j}_{hf}", name=f"yT{j}_{hf}")
        wout_sb = []
        bvwb = persist.tile([128, C], F32, name="bvwb")

        def dma_x(c0, w):
            for k in range(KT):
                nc.sync.dma_start(
                    out=xsb[:, k, c0:c0 + w],
                    in_=xT.rearrange("(k p) t -> p k t", p=128)[:, k, c0:c0 + w])

        # ---- per-group state ----
        gpools = {g: top.enter_context(tc.tile_pool(name=f"grp{g}", bufs=1))
                  for g in range(2)}
        # shared B-phase PSUM pools (both groups): opened before a_psum so
        # a_psum can close early (LIFO pool order)
        s_psum_pool = top.enter_context(
            tc.tile_pool(name="s_psum", bufs=2, space="PSUM"))
        y_psum_pool = top.enter_context(
            tc.tile_pool(name="y_psum", bufs=1, space="PSUM"))
        gstate = {}

        def setup_group(g):
            gpool = gpools[g]
            f0 = g * G_F
            wqk_sb = []
            for src, nm, m in [(wq, "q", 0), (wq, "q", 1), (wk, "k", 0), (wk, "k", 1)]:
                wt = gpool.tile([128, KT, 128], BF16, tag=f"w{nm}{m}",
                                name=f"w{nm}{g}_{m}")
                nc.sync.dma_start(
                    out=wt,
                    in_=src[:, f0 + m * 128:f0 + (m + 1) * 128].rearrange(
                        "(k p) f -> p k f", p=128))
                wqk_sb.append(wt)
            wv_sb = gpool.tile([128, KT, G_F], BF16, tag="wv", name=f"wv{g}")
            nc.sync.dma_start(
                out=wv_sb,
                in_=wv[:, f0:f0 + G_F].rearrange("(k p) f -> p k f", p=128))
            qT_sb = [gpool.tile([128, TT], qk_dt, tag=f"qT{j}", name=f"qT{g}_{j}")
                     for j in range(2)]
            kz_sb = [gpool.tile([128, TT], qk_dt, tag=f"kz{j}", name=f"kz{g}_{j}")
                     for j in range(2)]
            v_sb = gpool.tile([128, n_ttiles, G_HEADS, D + 1], BF16, tag="v",
                              name=f"v{g}")
            # fp8 copy of v; per-head slot padded 65->80 bytes so the
            # s-tile stride (4*80=320B) satisfies the DoubleRow ldweights
            # 16B-alignment rule
            v8_sb = gpool.tile([128, n_ttiles, G_HEADS, 80], FP8, tag="v8",
                               name=f"v8_{g}")
            nc.sync.dma_start(
                out=v_sb[:, :, :, D],
                in_=vones[:].rearrange("p (i h) -> p i h", h=G_HEADS))
            gstate[g] = dict(wqk=wqk_sb, wv=wv_sb, qT=qT_sb, kz=kz_sb, v=v_sb,
                             v8=v8_sb)

        a_psum_ctx = ExitStack()
        a_psum = a_psum_ctx.enter_context(
            tc.tile_pool(name="a_psum", bufs=2, space="PSUM"))

        def emit_a_qk(g, m, c0, ACH):
            st = gstate[g]
            ps = a_psum.tile([128, 512], F32, tag="mm", name="mmps")
            for k in range(KT):
                nc.tensor.matmul(
                    ps[:, :ACH],
                    st["wqk"][m][:, k, :],
                    xsb[:, k, c0:c0 + ACH],
                    start=(k == 0), stop=(k == KT - 1))
            bias = (bq_sb if m < 2 else bk_sb)[:, g * 2 + (m % 2):g * 2 + (m % 2) + 1]
            dst = (st["qT"] if m < 2 else st["kz"])[m % 2]
            nc.vector.tensor_scalar_add(
                dst[:, c0:c0 + ACH], ps[:, :ACH], bias)

        def emit_a_v(g, c0, sub):
            st = gstate[g]
            ps = a_psum.tile([128, 512], F32, tag="mm", name="mmps")
            for k in range(KT):
                nc.tensor.matmul(
                    ps[:, :G_F],
                    xsb[:, k, c0 + sub * 128:c0 + (sub + 1) * 128],
                    st["wv"][:, k, :],
                    start=(k == 0), stop=(k == KT - 1))
            it = c0 // 128 + sub
            nc.vector.tensor_copy(
                out=st["v"][:, it, :, 0:D],
                in_=ps[:, :G_F].rearrange("p (h d) -> p h d", h=G_HEADS))
            if it >= 4:  # fp8 copy (incl. ones col) for paired PV
                nc.gpsimd.tensor_copy(
                    out=st["v8"][:, it, :, 0:D + 1], in_=st["v"][:, it])

        def emit_a_chunk(g, c0, ACH):
            for m in range(4):  # 2 q-tiles + 2 k-tiles
                emit_a_qk(g, m, c0, ACH)
            for sub in range(ACH // 128):
                emit_a_v(g, c0, sub)

        # ---- S matmul helper (fp8 DoubleRow with stride-0 dup, or bf16) ----
        def emit_s(st, jt, po, s, c0, w, ps_ap):
            # ISA: matmul moving-operand free size <= 512 (DoubleRow: 2N<=512)
            lhsT = st["kz"][jt][po:po + 64, s * 128:(s + 1) * 128]
            if S_FP8:
                lhsT = lhsT.unsqueeze(1).broadcast_to([64, 2, 128])
                for o in range(0, w, 256):
                    pw = min(256, w - o)
                    rhs = st["qT"][jt][po:po + 64, c0 + o:c0 + o + pw]
                    nc.tensor.matmul(
                        ps_ap[:, o:o + pw],
                        lhsT,
                        rhs.unsqueeze(1).broadcast_to([64, 2, pw]),
                        start=True, stop=True,
                        perf_mode=mybir.MatmulPerfMode.DoubleRow,
                        skip_group_check=True)
            else:
                for o in range(0, w, 512):
                    pw = min(512, w - o)
                    nc.tensor.matmul(
                        ps_ap[:, o:o + pw],
                        lhsT,
                        st["qT"][jt][po:po + 64, c0 + o:c0 + o + pw],
                        start=True, stop=True,
                        skip_group_check=True)

        # ---- attention unit.  s-tiles < PV8_S0 run bf16 singles; s-tiles
        # >= PV8_S0 run as fp8 DoubleRow pairs (2 s-tiles per PV matmul).
        PV8_S0 = 4

        def emit_unit(g, hh, half, filler=None):
            st = gstate[g]
            jt = hh // 2
            po = 64 * (hh % 2)
            h_local = g * G_HEADS + hh
            h0 = half * HW
            y_ps = y_psum_pool.tile([D + 1, HW], F32, tag="y", name="y_ps")
            n_s = min(n_ttiles, (h0 + HW) // 128)

            LAG = 3
            pend = []

            def slot_done(pv_closure):
                pend.append(pv_closure)
                if len(pend) > LAG:
                    pend.pop(0)()
                if filler is not None:
                    filler()

            # bf16 singles
            for s in range(min(PV8_S0, n_s)):
                t0 = max(s * 128, h0)
                for (c0, w) in _chunks_for_s(t0, h0 + HW):
                    s_ps = s_psum_pool.tile([128, 1024], F32, tag="s",
                                            name="s_ps")
                    emit_s(st, jt, po, s, c0, w, s_ps[:, :w])
                    diag = (c0 == s * 128)
                    if diag and s % 2 == 0:  # pre-exp additive mask (DVE)
                        nc.vector.tensor_add(
                            s_ps[:, 0:128], s_ps[:, 0:128], trineg_sb)
                    u_sb = u_pool.tile([128, 1024], BF16, tag="u", name="u_sb")
                    nc.scalar.activation(
                        out=u_sb[:, :w], in_=s_ps[:, :w],
                        func=mybir.ActivationFunctionType.Exp,
                        scale=s_scale)
                    if diag and s % 2 == 1:  # post-exp mul mask (GpSimd)
                        nc.gpsimd.tensor_mul(
                            u_sb[:, 0:128], u_sb[:, 0:128], tri_sb)

                    def pv1(s=s, c0=c0, w=w, u_sb=u_sb):
                        for o in range(0, w, 512):
                            pw = min(512, w - o)
                            nc.tensor.matmul(
                                y_ps[:, c0 - h0 + o:c0 - h0 + o + pw],
                                st["v"][:, s, hh, :],
                                u_sb[:, o:o + pw],
                                start=(s == 0), stop=False,
                                skip_group_check=True)
                    slot_done(pv1)

            # fp8 DoubleRow pairs
            for s in range(PV8_S0, n_s, 2):
                t0 = max(s * 128, h0)
                for (c0, w) in _chunks_for_s(t0, h0 + HW):
                    diag = (c0 == s * 128)
                    u8 = u_pool.tile([128, 2, 1024], FP8, tag="u8", name="u8")
                    ps_a = s_psum_pool.tile([128, 1024], F32, tag="s",
                                            name="s_psa")
                    emit_s(st, jt, po, s, c0, w, ps_a[:, :w])
                    ps_b = s_psum_pool.tile([128, 1024], F32, tag="s",
                                            name="s_psb")
                    if diag:
                        # row s+1: cols [0:128) fully masked -> zero-fill u8
                        emit_s(st, jt, po, s + 1, c0 + 128, w - 128,
                               ps_b[:, 128:w])
                        nc.gpsimd.memset(u8[:, 1, 0:128], 0.0)
                        nc.scalar.activation(
                            out=u8[:, 0, :w], in_=ps_a[:, :w],
                            func=mybir.ActivationFunctionType.Exp,
                            scale=s_scale)
                        nc.scalar.activation(
                            out=u8[:, 1, 128:w], in_=ps_b[:, 128:w],
                            func=mybir.ActivationFunctionType.Exp,
                            scale=s_scale)
                        # post-exp triangular masks on the fp8 u (cheap 1B DVE)
                        nc.vector.tensor_mul(
                            u8[:, 0, 0:128], u8[:, 0, 0:128], tri8_sb)
                        nc.vector.tensor_mul(
                            u8[:, 1, 128:256], u8[:, 1, 128:256], tri8_sb)
                    else:
                        emit_s(st, jt, po, s + 1, c0, w, ps_b[:, :w])
                        nc.scalar.activation(
                            out=u8[:, 0, :w], in_=ps_a[:, :w],
                            func=mybir.ActivationFunctionType.Exp,
                            scale=s_scale)
                        nc.scalar.activation(
                            out=u8[:, 1, :w], in_=ps_b[:, :w],
                            func=mybir.ActivationFunctionType.Exp,
                            scale=s_scale)

                    def pv2(s=s, c0=c0, w=w, u8=u8):
                        for o in range(0, w, 256):
                            pw = min(256, w - o)
                            nc.tensor.matmul(
                                y_ps[:, c0 - h0 + o:c0 - h0 + o + pw],
                                st["v8"][:, s:s + 2, hh, 0:D + 1],
                                u8[:, :, o:o + pw],
                                start=False, stop=False,
                                perf_mode=mybir.MatmulPerfMode.DoubleRow,
                                skip_group_check=True)
                    slot_done(pv2)

            for pv in pend:
                pv()

            # stash unscaled y; eager per-half normalization
            yT_dst = yT_sb[(h_local // 2, half)][po:po + 64, :]
            nc.vector.tensor_copy(out=yT_dst, in_=y_ps[0:D, :])
            rz = z_pool.tile([1, HW], F32, tag="rz", name="rz", bufs=2)
            nc.vector.reciprocal(rz, y_ps[D:D + 1, :])
            rzb = z_pool.tile([128, HW], F32, tag="rzb", name="rzb", bufs=2)
            nc.gpsimd.partition_broadcast(rzb, rz)
            nc.vector.tensor_mul(yT_dst, yT_dst, rzb[po:po + 64, :])

        # ---- out projection ----
        cpools = {}

        def emit_c_mtiles(ms):
            if "psum" not in cpools:
                cpools["psum"] = top.enter_context(
                    tc.tile_pool(name="c_psum", bufs=2, space="PSUM"))
                cpools["outbuf"] = top.enter_context(
                    tc.tile_pool(name="outbuf", bufs=3))
            c_psum, outbuf = cpools["psum"], cpools["outbuf"]
            mw = HW // 128
            for m in ms:
                ob = outbuf.tile([128, C], F32, tag="ob", name="ob")
                m_half, m_off = divmod(m, mw)
                for n in range(C // 512):
                    ps = c_psum.tile([128, 512], F32, tag="mm", name="mmps_c")
                    for j in range(FL // 128):
                        nc.tensor.matmul(
                            ps,
                            yT_sb[(j, m_half)][:, m_off * 128:(m_off + 1) * 128],
                            wout_sb[j][:, n * 512:(n + 1) * 512],
                            start=(j == 0), stop=(j == FL // 128 - 1))
                    nc.vector.tensor_add(
                        ob[:, n * 512:(n + 1) * 512], ps,
                        bvwb[:, n * 512:(n + 1) * 512])
                nc.sync.dma_start(out=out[m * 128:(m + 1) * 128, :], in_=ob)

        # ================= schedule =================
        # upfront: x for t<1024, group-0 weights, consts; A0 over t<1024
        dma_x(0, 128)
        setup_group(0)
        nc.sync.dma_start(out=bq_sb, in_=bq[:].rearrange("(m p) -> p m", p=128))
        nc.sync.dma_start(out=bk_sb, in_=bk[:].rearrange("(m p) -> p m", p=128))
        dma_x(128, 128)
        nc.sync.dma_start(out=tri_sb, in_=tri[:])
        nc.sync.dma_start(out=tri8_sb, in_=tri8[:])
        nc.sync.dma_start(out=trineg_sb, in_=trineg[:])
        nc.sync.dma_start(out=bvw_sb, in_=bvw[:])
        dma_x(256, 256)
        dma_x(512, 512)
        dma_x(1024, 512)
        dma_x(1536, 512)
        for (c0, w) in [(0, 128), (128, 128), (256, 256), (512, 512)]:
            emit_a_chunk(0, c0, w)

        # group-1 weights go out on the wire during group-0 attention
        setup_group(1)
        # w_out + bvw broadcast early too
        for j in range(FL // 128):
            wt = persist.tile([128, C], BF16, tag=f"wout{j}", name=f"wout{j}")
            nc.sync.dma_start(out=wt, in_=wout[j * 128:(j + 1) * 128, :])
            wout_sb.append(wt)
        nc.gpsimd.partition_broadcast(bvwb, bvw_sb)

        # ---- quantum fill system: small PE work items interleaved into the
        # exp-paced attention units.  (avail, deadline, closure); both in
        # unit indices.  Flush anything past deadline before its unit.
        quanta = []

        def add_a_quanta(g, chunks, avail, deadline):
            for (c0, w) in chunks:
                for m in range(4):
                    quanta.append((avail, deadline,
                                   lambda g=g, m=m, c0=c0, w=w:
                                   emit_a_qk(g, m, c0, w)))
                for sub in range(w // 128):
                    quanta.append((avail, deadline,
                                   lambda g=g, c0=c0, sub=sub:
                                   emit_a_v(g, c0, sub)))

        add_a_quanta(0, [(1024, 512), (1536, 512)], 0, 4)
        add_a_quanta(1, [(0, 512), (512, 512)], 0, 8)
        add_a_quanta(1, [(1024, 512), (1536, 512)], 4, 12)

        qi = 0
        cur_ui = 0

        def filler():
            nonlocal qi
            if qi < len(quanta) and quanta[qi][0] <= cur_ui:
                quanta[qi][2]()
                qi += 1

        units = []
        for g in range(2):
            for half in range(n_halves):
                for hh in range(G_HEADS):
                    units.append((g, hh, half))

        for ui, (g, hh, half) in enumerate(units):
            cur_ui = ui
            while qi < len(quanta) and quanta[qi][1] <= ui:
                quanta[qi][2]()
                qi += 1
            if ui == 12:
                a_psum_ctx.close()
                for m in range(8):
                    quanta.append((12, 16,
                                   lambda m=m: emit_c_mtiles([m])))
            emit_unit(g, hh, half, filler=filler)
        while qi < len(quanta):
            quanta[qi][2]()
            qi += 1
        emit_c_mtiles(range(8, 16))

    nc.compile()
    return nc


_CACHED = {}


def _get_program():
    if "nc" not in _CACHED:
        _CACHED["nc"] = build_program()
    return _CACHED["nc"]


def prepare_in_maps(x, w_qkv, b_qkv, w_out):
    import ml_dtypes
    bf16 = ml_dtypes.bfloat16
    in_maps = []
    for core in range(N_CORES):
        b = core // 2
        g = core % 2
        qs, ks, vs = g * FL, C + g * FL, 2 * C + g * FL
        bv_local = b_qkv[vs:vs + FL].astype(np.float32)
        wout_local = w_out[g * FL:(g + 1) * FL, :].astype(np.float32)
        bvw = (bv_local @ wout_local).reshape(1, C).astype(np.float32)
        in_maps.append({
            "xT": np.ascontiguousarray(x[b].T).astype(bf16),
            "wq": np.ascontiguousarray(w_qkv[:, qs:qs + FL]).astype(bf16),
            "wk": np.ascontiguousarray(w_qkv[:, ks:ks + FL]).astype(bf16),
            "wv": np.ascontiguousarray(w_qkv[:, vs:vs + FL]).astype(bf16),
            "wout": np.ascontiguousarray(wout_local).astype(bf16),
            "bq": np.ascontiguousarray(b_qkv[qs:qs + FL]).astype(np.float32),
            "bk": np.ascontiguousarray(b_qkv[ks:ks + FL]).astype(np.float32),
            "bvw": bvw,
            "tri": np.triu(np.ones((128, 128), dtype=np.float32)).astype(bf16),
            "tri8": np.triu(np.ones((128, 128), dtype=np.float32)).astype(
                ml_dtypes.float8_e4m3fn),
            "trineg": (np.tril(np.ones((128, 128), dtype=np.float32), -1)
                       * np.float32(-1e9)),
            "vones": np.ones((128, (T // 128) * G_HEADS), dtype=np.float32).astype(bf16),
        })
    return in_maps


def gather(results, b_out):
    out = np.empty((B, T, C), dtype=np.float32)
    for b in range(B):
        out[b] = (results[2 * b]["out"].astype(np.float32)
                  + results[2 * b + 1]["out"].astype(np.float32) + b_out)
    return out


def kernel(x, w_qkv, b_qkv, w_out, b_out):
    x = np.asarray(x, dtype=np.float32)
    w_qkv = np.asarray(w_qkv, dtype=np.float32)
    b_qkv = np.asarray(b_qkv, dtype=np.float32)
    w_out = np.asarray(w_out, dtype=np.float32)
    b_out = np.asarray(b_out, dtype=np.float32)

    nc = _get_program()
    in_maps = prepare_in_maps(x, w_qkv, b_qkv, w_out)
    res = run_bass_kernel_spmd(nc, in_maps, core_ids=list(range(N_CORES)))
    return gather(res.results, b_out)


if __name__ == "__main__":
    rng = np.random.default_rng(0)
    inputs = {
        "x": rng.standard_normal((B, T, C), dtype=np.float32),
        "w_qkv": rng.standard_normal((C, 3 * C), dtype=np.float32) * 0.02,
        "b_qkv": np.zeros((3 * C,), dtype=np.float32),
        "w_out": rng.standard_normal((C, C), dtype=np.float32) * 0.02,
        "b_out": np.zeros((C,), dtype=np.float32),
    }
    y = kernel(**inputs)
    print("ok", y.shape, y.dtype)
